# revision 53
# baseline (speedup 1.0000x reference)
"""Mixtral sparse MoE block on 8 TRN2 NeuronCores.

Strategy (expert-parallel, per sharding hint):
  - Router (tiny: 2048x1024 @ 1024x8 + softmax + top-2) runs on host as part
    of the sharding step; it determines which tokens go to which core.
  - Core e holds expert e's weights (w1/w2/w3 in fp16, 22 MB) and receives
    the tokens routed to expert e (zero-padded to a static capacity C),
    pre-transposed and cast to fp16.
  - Device computes hidT = silu(W1 x^T) * (W3 x^T); outT = W2 hidT, i.e. the
    full SwiGLU MLP in transposed layout. fp16 operands, fp32 PSUM
    accumulation (matmul rate is identical to fp32r at N=512, but weight
    DMA and ldweights traffic halve).
  - Host scales each expert output row by its routing weight and scatter-adds
    back into the [T, H] output.

Shapes are hardcoded for the graded problem:
  hidden_states [1, 2048, 1024], gate_w [8, 1024],
  w1/w3 [8, 3584, 1024], w2 [8, 1024, 3584], fp32.
"""

import os

import numpy as np

import concourse.bass as bass
import concourse.tile as tile
from concourse import mybir
from concourse.bass_utils import run_bass_kernel_spmd

E = 8          # experts == cores
TOP_K = 2
H = 1024       # hidden
I = 3584       # intermediate
T = 2048       # tokens
P = 128
NH = H // P    # 8
NI = I // P    # 28
C = 512        # per-expert token capacity; overflow tokens go to the host path
CT = 512       # matmul N-tile == C: one full-width matmul per group
NCT = C // CT

F32 = mybir.dt.float32
F16 = mybir.dt.float16

_cache = {}


def _build_moe_mlp():
    """One-expert SwiGLU MLP, SPMD on 8 cores.

    Inputs (per core, host pre-arranged, fp16):
      xP   [P, NH*C]       tokens^T partition-packed: xP[p, hc*C+c] =
                           x[c-th routed token][hc*P+p]. One wide SBUF tile
                           (8 KB per partition line) so the DMA runs at
                           queue bandwidth instead of the 1KB-packet rate.
      w13p [NI/2, P, 2*2*NH*P] w1 and w3 block-packed, TWO ic blocks per
                           DRAM row (8KB lines — a 4KB-line stream loses the
                           saturated-fabric packet arbitration and starves):
                           w13p[ic//2, hp, (ic%2)*2*NH*P + h*P+ip] =
                             w1[ic*P+ip, h*P+hp]; w3 likewise at +NH*P.
      w2c  [NH, P, NI*P]   w2c[hc, ip, ic*P+hp] = w2[hc*P+hp, ic*P+ip]
    Output:
      outT [H, C] = (silu(x@w1.T) * (x@w3.T)) @ w2.T, transposed, fp32
    """
    nc = bass.Bass(use_seq_codegen=True)
    xP = nc.declare_dram_parameter("xP", [P, NH * C], F16, isOutput=False)
    w13p = nc.declare_dram_parameter(
        "w13p", [NI // 2, P, 4 * NH * P], F16, isOutput=False
    )
    w2c = nc.declare_dram_parameter(
        "w2c", [NH // 2, P, 2 * NI * P], F16, isOutput=False
    )
    outT = nc.declare_dram_parameter("outT", [H, C], F32, isOutput=True)

    with tile.TileContext(nc) as tc:
        with (
            tc.tile_pool(name="x_pool", bufs=1) as x_pool,
            tc.tile_pool(name="hid_pool", bufs=1) as hid_pool,
            tc.tile_pool(name="w13_pool", bufs=4) as w13_pool,
            tc.tile_pool(name="w2_pool", bufs=1) as w2_pool,
            tc.tile_pool(name="ps1", bufs=3, space="PSUM") as ps1,
            tc.tile_pool(name="ps3", bufs=3, space="PSUM") as ps3,
            tc.tile_pool(name="pso", bufs=2, space="PSUM") as pso,
            tc.tile_pool(name="act_pool", bufs=3) as act_pool,
            tc.tile_pool(name="out_pool", bufs=3) as out_pool,
        ):
            # Stage 0: token activations as ONE wide tile [P, NH*C].
            # DMA arbitration is per-PACKET (one partition line) round-robin
            # across queues, so throughput is proportional to line size:
            # keep every line >= 4KB. x: two half DMAs (4KB lines).
            # Startup: x quarters alternate the two HW queues in consumption
            # order; ic0 streams as four quarter-DMAs on gpsimd so the first
            # p1 chain only waits for ic0's w1 block, then ic1 follows as
            # one 4KB-line single.
            W = 2 * NH * P  # w13 columns per ic block
            p0 = w13_pool.tile([P, 2 * W], F16, tag="w13", name="w13p0")
            w13_tiles = {0: p0}

            xt = x_pool.tile([P, NH * C], F16, tag="x", name="x")
            XQ = NH * C // 4
            for q in range(4):
                eng = nc.sync if q % 2 == 0 else nc.scalar
                eng.dma_start(
                    out=xt[:, bass.ds(q * XQ, XQ)],
                    in_=xP[:, bass.ds(q * XQ, XQ)],
                )
            for qq in range(2):
                nc.gpsimd.dma_start(
                    out=p0[:, bass.ds(qq * W // 2, W // 2)],
                    in_=w13p[0, :, bass.ds(qq * W // 2, W // 2)],
                )
            nc.gpsimd.dma_start(out=p0[:, W:], in_=w13p[0, :, W:])
            # pair 1 (ic2, ic3) as two 4KB-line singles on scalar so the
            # early stream arrives at finer granularity
            p1t = w13_pool.tile([P, 2 * W], F16, tag="w13", name="w13p1")
            w13_tiles[1] = p1t
            nc.scalar.dma_start(out=p1t[:, :W], in_=w13p[1, :, :W])
            nc.scalar.dma_start(out=p1t[:, W:], in_=w13p[1, :, W:])


            # Queue plan (total DRAM BW ~350 GB/s is SHARED): the w13 stream
            # needs ~147 GB/s sustained and gets the fast software-DGE queue
            # (gpsimd, ~200 GB/s) to itself. x (1 MB now) and the w2
            # prefetch (7.3 MB, needed only at ~100us) share the two slower
            # HW queues (~70 GB/s each) — w2 lands by ~65us.
            # double-wide tiles (2 hc blocks per row): halves the DMA and
            # semaphore count, lines stay 7KB
            w2_sb = []
            for hg in range(NH // 2):
                w2t = w2_pool.tile(
                    [P, 2 * NI * P], F16, tag=f"w2_{hg}", name=f"w2_{hg}"
                )
                nc.sync.dma_start(out=w2t[:], in_=w2c[hg, :, :])
                w2_sb.append(w2t)

            # hidT [I, C] lives in SBUF between the two stages (fp16).
            hid_sb = [
                hid_pool.tile([P, C], F16, tag=f"hid{ic}", name=f"hid{ic}") for ic in range(NI)
            ]

            # Stage 1: hidT[ic] = silu(p1) * p3, contracting over H.
            # w13 streams as one 8KB-line DMA per ic PAIR, pairs alternating
            # between gpsimd and scalar: the two queues together earn ~2/3
            # of the byte share, which the ~150 GB/s stream needs; sync
            # carries x + w2.
            for ic in range(NI):
                if ic % 2 == 0:
                    icp = ic // 2
                    if icp > 1:
                        w13t = w13_pool.tile([P, 2 * W], F16, tag="w13")
                        w13_tiles[icp] = w13t
                        eng = nc.gpsimd if icp % 2 == 0 else nc.scalar
                        eng.dma_start(out=w13t[:], in_=w13p[icp, :, :])
                    else:
                        w13t = w13_tiles[icp]
                else:
                    w13t = w13_tiles[ic // 2]
                base = (ic % 2) * W
                w1t = w13t[:, bass.ds(base, NH * P)]
                w3t = w13t[:, bass.ds(base + NH * P, NH * P)]
                for nt in range(NCT):
                    cs = bass.ts(nt, CT)
                    p1 = ps1.tile([P, CT], F32, tag="p1")
                    p3 = ps3.tile([P, CT], F32, tag="p3")
                    for hc in range(NH):
                        nc.tensor.matmul(
                            p1[:],
                            lhsT=w1t[:, bass.ts(hc, P)],
                            rhs=xt[:, bass.ds(hc * C + nt * CT, CT)],
                            start=(hc == 0),
                            stop=(hc == NH - 1),
                        )
                    for hc in range(NH):
                        nc.tensor.matmul(
                            p3[:],
                            lhsT=w3t[:, bass.ts(hc, P)],
                            rhs=xt[:, bass.ds(hc * C + nt * CT, CT)],
                            start=(hc == 0),
                            stop=(hc == NH - 1),
                        )
                    # Balance the PSUM eviction: ACT does silu(p1), DVE
                    # copies p3 (one PSUM port, one PE wait) — ACT alone
                    # otherwise releases every PSUM slot and runs late.
                    # The two-wait mul is handled by _split_excess_waits.
                    s1 = act_pool.tile([P, CT], F16, tag="s1")
                    nc.scalar.activation(
                        s1[:], p1[:], mybir.ActivationFunctionType.Silu
                    )
                    c3 = act_pool.tile([P, CT], F16, tag="c3")
                    nc.vector.tensor_copy(c3[:], p3[:])
                    nc.vector.tensor_mul(hid_sb[ic][:, cs], s1[:], c3[:])

            # Stage 2: outT[hc] = w2 @ hid, contracting over I.
            for hc in range(NH):
                w2t = w2_sb[hc // 2]
                w2base = (hc % 2) * NI * P
                for nt in range(NCT):
                    cs = bass.ts(nt, CT)
                    po = pso.tile([P, CT], F32, tag="po")
                    for ic in range(NI):
                        nc.tensor.matmul(
                            po[:],
                            lhsT=w2t[:, bass.ds(w2base + ic * P, P)],
                            rhs=hid_sb[ic][:, cs],
                            start=(ic == 0),
                            stop=(ic == NI - 1),
                        )
                    # Evict split by PARTITION halves: two copies (DVE +
                    # ACT) and two 2KB-line DMAs (sync + scalar) all run in
                    # parallel, halving the exposed tail after the final
                    # matmul. Column splits would shrink the DMA lines below
                    # 2KB and hit the packet-rate wall instead.
                    # Single full-width eviction (fp32 rows = 2KB DMA lines);
                    # finer splits add end-of-program semaphore barriers that
                    # cost more than the overlap they buy.
                    ot = out_pool.tile([P, CT], F32, tag="ot")
                    nc.vector.tensor_copy(ot[:], po[:])
                    eng = nc.sync if hc % 2 == 0 else nc.scalar
                    eng.dma_start(
                        out=outT[hc * P:(hc + 1) * P, bass.ds(nt * CT, CT)],
                        in_=ot[:],
                    )
    _split_excess_waits(nc)
    return nc


def _split_excess_waits(nc, max_inline=1):
    """This walrus build rejects instructions carrying more than one inline
    sem wait ("Too many sync wait commands"). Move excess on_wait entries
    onto standalone InstEventSemaphore ops right before the instruction on
    the same engine (semantically identical: the engine stalls either way).
    """
    for blk in nc.m.functions[0].blocks:
        insts = blk.instructions
        out = []
        changed = False
        for inst in insts:
            si = inst.sync_info
            waits = list(si.on_wait) if si is not None and si.on_wait else []
            if len(waits) > max_inline and not isinstance(
                inst, mybir.InstEventSemaphore
            ):
                excess, keep = waits[:-max_inline], waits[-max_inline:]
                for k, w in enumerate(excess):
                    out.append(
                        mybir.InstEventSemaphore(
                            name=f"{inst.name}-evw{k}",
                            engine=inst.engine,
                            sync_info=mybir.SyncInfo(on_wait=[w], on_update=[]),
                        )
                    )
                inst.sync_info = mybir.SyncInfo(
                    on_wait=keep, on_update=list(si.on_update or [])
                )
                changed = True
            out.append(inst)
        if changed:
            blk.instructions = out
    return nc


def _route(x, gate_w):
    """Replicate the reference router in f64-stable numpy: returns
    (top_idx [T,K], top_w [T,K]) with renormalized weights."""
    logits = x.astype(np.float64) @ gate_w.astype(np.float64).T  # [T, E]
    m = logits.max(axis=-1, keepdims=True)
    p = np.exp(logits - m)
    p /= p.sum(axis=-1, keepdims=True)
    # top-2, ties broken by lower index (matches jax.lax.top_k)
    order = np.argsort(-p, axis=-1, kind="stable")
    top_i = order[:, :TOP_K]
    top_p = np.take_along_axis(p, top_i, axis=-1)
    top_w = top_p / top_p.sum(axis=-1, keepdims=True)
    return top_i, top_w.astype(np.float32)


def kernel(hidden_states, gate_w, w1, w2, w3):
    b, s, h = hidden_states.shape
    x = np.ascontiguousarray(
        np.asarray(hidden_states, dtype=np.float32).reshape(-1, h)
    )
    gate_w = np.asarray(gate_w, dtype=np.float32)
    w1 = np.asarray(w1, dtype=np.float32)
    w2 = np.asarray(w2, dtype=np.float32)
    w3 = np.asarray(w3, dtype=np.float32)

    top_i, top_w = _route(x, gate_w)

    # token lists per expert
    expert_rows = [np.where((top_i == e).any(axis=1))[0] for e in range(E)]
    # (row in expert buffer) for each (token, k) assignment
    in_maps = []
    overflow = []  # (e, token_idx, weight) handled on host
    gathers = []
    for e in range(E):
        rows = expert_rows[e]
        if len(rows) > C:
            keep = rows[:C]
            for t in rows[C:]:
                kk = np.where(top_i[t] == e)[0][0]
                overflow.append((e, int(t), float(top_w[t, kk])))
            rows = keep
        gathers.append(rows)
        xe = np.zeros((C, H), dtype=np.float32)
        xe[: len(rows)] = x[rows]
        # [P, NH*C]: xP[p, hc*C+c] = xe[c, hc*P+p]
        xP = np.ascontiguousarray(
            xe.T.reshape(NH, P, C).transpose(1, 0, 2).reshape(P, NH * C)
        ).astype(np.float16)
        w1c = w1[e].reshape(NI, P, NH, P).transpose(0, 3, 2, 1).reshape(NI, P, NH * P)
        w3c = w3[e].reshape(NI, P, NH, P).transpose(0, 3, 2, 1).reshape(NI, P, NH * P)
        w13c = np.concatenate([w1c, w3c], axis=2)  # [NI, P, 2*NH*P]
        # pair-pack: two consecutive ic blocks share a DRAM row (8KB lines)
        w13p = np.ascontiguousarray(
            w13c.reshape(NI // 2, 2, P, 2 * NH * P)
            .transpose(0, 2, 1, 3)
            .reshape(NI // 2, P, 4 * NH * P)
        ).astype(np.float16)
        w2c = (
            w2[e].reshape(NH, P, NI, P).transpose(0, 3, 2, 1).reshape(NH, P, NI * P)
        )
        # two hc blocks per DRAM row (halves DMA count, lines stay 7KB)
        w2c = np.ascontiguousarray(
            w2c.reshape(NH // 2, 2, P, NI * P)
            .transpose(0, 2, 1, 3)
            .reshape(NH // 2, P, 2 * NI * P)
        ).astype(np.float16)
        in_maps.append({"xP": xP, "w13p": w13p, "w2c": w2c})

    if "nc" not in _cache:
        _cache["nc"] = _build_moe_mlp()
    nc = _cache["nc"]

    res = run_bass_kernel_spmd(
        nc,
        in_maps,
        core_ids=list(range(E)),
        trace=bool(int(os.environ.get("MOE_TRACE", "0"))),
    )
    _cache["last_result"] = res

    out = np.zeros((T, H), dtype=np.float32)
    for e in range(E):
        rows = gathers[e]
        ye = np.ascontiguousarray(res.results[e]["outT"].T)[: len(rows)]  # [n_e, H]
        # routing weight of expert e for each routed token
        kidx = (top_i[rows] == e).argmax(axis=1)
        wts = top_w[rows, kidx][:, None]
        np.add.at(out, rows, ye * wts)

    if overflow:
        from collections import defaultdict
        by_e = defaultdict(list)
        for e, t, wt in overflow:
            by_e[e].append((t, wt))
        for e, lst in by_e.items():
            ts = np.array([t for t, _ in lst])
            wts = np.array([w for _, w in lst], dtype=np.float32)[:, None]
            xb = x[ts]
            hid = _silu_np(xb @ w1[e].T) * (xb @ w3[e].T)
            np.add.at(out, ts, wts * (hid @ w2[e].T))

    return out.reshape(b, s, h)


def _silu_np(v):
    return v / (1.0 + np.exp(-v))


# revision 54
# speedup vs baseline: 1.1498x; 1.1498x over previous
"""Mixtral sparse MoE block on 8 TRN2 NeuronCores.

Strategy (expert-parallel, per sharding hint):
  - Router (tiny: 2048x1024 @ 1024x8 + softmax + top-2) runs on host as part
    of the sharding step; it determines which tokens go to which core.
  - Core e holds expert e's weights (w1/w2/w3 in fp16, 22 MB) and receives
    the tokens routed to expert e (zero-padded to a static capacity C),
    pre-transposed and cast to fp16.
  - Device computes hidT = silu(W1 x^T) * (W3 x^T); outT = W2 hidT, i.e. the
    full SwiGLU MLP in transposed layout. fp16 operands, fp32 PSUM
    accumulation (matmul rate is identical to fp32r at N=512, but weight
    DMA and ldweights traffic halve).
  - Host scales each expert output row by its routing weight and scatter-adds
    back into the [T, H] output.

Shapes are hardcoded for the graded problem:
  hidden_states [1, 2048, 1024], gate_w [8, 1024],
  w1/w3 [8, 3584, 1024], w2 [8, 1024, 3584], fp32.
"""

import os

import numpy as np

import concourse.bass as bass
import concourse.tile as tile
from concourse import mybir
from concourse.bass_utils import run_bass_kernel_spmd

E = 8          # experts == cores
TOP_K = 2
H = 1024       # hidden
I = 3584       # intermediate
T = 2048       # tokens
P = 128
NH = H // P    # 8
NI = I // P    # 28
C = 512        # per-expert token capacity; overflow tokens go to the host path
CT = 512       # matmul N-tile == C: one full-width matmul per group
NCT = C // CT

F32 = mybir.dt.float32
F16 = mybir.dt.float16

_cache = {}


def _build_moe_mlp():
    """One-expert SwiGLU MLP, SPMD on 8 cores.

    Inputs (per core, host pre-arranged, fp16):
      xP   [P, NH*C]       tokens^T partition-packed: xP[p, hc*C+c] =
                           x[c-th routed token][hc*P+p]. One wide SBUF tile
                           (8 KB per partition line) so the DMA runs at
                           queue bandwidth instead of the 1KB-packet rate.
      w13p [NI/2, P, 2*2*NH*P] w1 and w3 block-packed, TWO ic blocks per
                           DRAM row (8KB lines — a 4KB-line stream loses the
                           saturated-fabric packet arbitration and starves):
                           w13p[ic//2, hp, (ic%2)*2*NH*P + h*P+ip] =
                             w1[ic*P+ip, h*P+hp]; w3 likewise at +NH*P.
      w2c  [NH, P, NI*P]   w2c[hc, ip, ic*P+hp] = w2[hc*P+hp, ic*P+ip]
    Output:
      outT [H, C] = (silu(x@w1.T) * (x@w3.T)) @ w2.T, transposed, fp32
    """
    nc = bass.Bass(use_seq_codegen=True)
    xP = nc.declare_dram_parameter("xP", [P, NH * C], F16, isOutput=False)
    w13p = nc.declare_dram_parameter(
        "w13p", [NI // 2, P, 4 * NH * P], F16, isOutput=False
    )
    w2c = nc.declare_dram_parameter(
        "w2c", [NH // 2, P, 2 * NI * P], F16, isOutput=False
    )
    outT = nc.declare_dram_parameter("outT", [H, C], F32, isOutput=True)

    with tile.TileContext(nc) as tc:
        with (
            tc.tile_pool(name="x_pool", bufs=1) as x_pool,
            tc.tile_pool(name="hid_pool", bufs=1) as hid_pool,
            tc.tile_pool(name="w13_pool", bufs=4) as w13_pool,
            tc.tile_pool(name="w2_pool", bufs=1) as w2_pool,
            tc.tile_pool(name="ps1", bufs=2, space="PSUM") as ps1,
            tc.tile_pool(name="ps3", bufs=2, space="PSUM") as ps3,
            tc.tile_pool(name="pso", bufs=2, space="PSUM") as pso,
            tc.tile_pool(name="act_pool", bufs=3) as act_pool,
            tc.tile_pool(name="out_pool", bufs=3) as out_pool,
        ):
            # Stage 0: token activations as ONE wide tile [P, NH*C].
            # DMA arbitration is per-PACKET (one partition line) round-robin
            # across queues, so throughput is proportional to line size:
            # keep every line >= 4KB. x: two half DMAs (4KB lines).
            # Startup: x quarters alternate the two HW queues in consumption
            # order; ic0 streams as four quarter-DMAs on gpsimd so the first
            # p1 chain only waits for ic0's w1 block, then ic1 follows as
            # one 4KB-line single.
            W = 2 * NH * P  # w13 columns per ic block
            p0 = w13_pool.tile([P, 2 * W], F16, tag="w13", name="w13p0")
            w13_tiles = {0: p0}

            xt = x_pool.tile([P, NH * C], F16, tag="x", name="x")
            XQ = NH * C // 4
            for q in range(4):
                eng = nc.sync if q % 2 == 0 else nc.scalar
                eng.dma_start(
                    out=xt[:, bass.ds(q * XQ, XQ)],
                    in_=xP[:, bass.ds(q * XQ, XQ)],
                )
            for qq in range(2):
                nc.gpsimd.dma_start(
                    out=p0[:, bass.ds(qq * W // 2, W // 2)],
                    in_=w13p[0, :, bass.ds(qq * W // 2, W // 2)],
                )
            nc.gpsimd.dma_start(out=p0[:, W:], in_=w13p[0, :, W:])
            # pair 1 (ic2, ic3) as two 4KB-line singles on scalar so the
            # early stream arrives at finer granularity
            p1t = w13_pool.tile([P, 2 * W], F16, tag="w13", name="w13p1")
            w13_tiles[1] = p1t
            nc.scalar.dma_start(out=p1t[:, :W], in_=w13p[1, :, :W])
            nc.scalar.dma_start(out=p1t[:, W:], in_=w13p[1, :, W:])


            # Queue plan (total DRAM BW ~350 GB/s is SHARED): the w13 stream
            # needs ~147 GB/s sustained and gets the fast software-DGE queue
            # (gpsimd, ~200 GB/s) to itself. x (1 MB now) and the w2
            # prefetch (7.3 MB, needed only at ~100us) share the two slower
            # HW queues (~70 GB/s each) — w2 lands by ~65us.
            # double-wide tiles (2 hc blocks per row): halves the DMA and
            # semaphore count, lines stay 7KB
            w2_sb = []
            for hg in range(NH // 2):
                w2t = w2_pool.tile(
                    [P, 2 * NI * P], F16, tag=f"w2_{hg}", name=f"w2_{hg}"
                )
                nc.sync.dma_start(out=w2t[:], in_=w2c[hg, :, :])
                w2_sb.append(w2t)

            # hidT [I, C] lives in SBUF between the two stages (fp16).
            hid_sb = [
                hid_pool.tile([P, C], F16, tag=f"hid{ic}", name=f"hid{ic}") for ic in range(NI)
            ]

            # Stage 1: hidT[ic] = silu(p1) * p3, contracting over H.
            # w13 streams as one 8KB-line DMA per ic PAIR, pairs alternating
            # between gpsimd and scalar: the two queues together earn ~2/3
            # of the byte share, which the ~150 GB/s stream needs; sync
            # carries x + w2.
            for ic in range(NI):
                if ic % 2 == 0:
                    icp = ic // 2
                    if icp > 1:
                        w13t = w13_pool.tile([P, 2 * W], F16, tag="w13")
                        w13_tiles[icp] = w13t
                        eng = nc.gpsimd if icp % 2 == 0 else nc.scalar
                        eng.dma_start(out=w13t[:], in_=w13p[icp, :, :])
                    else:
                        w13t = w13_tiles[icp]
                else:
                    w13t = w13_tiles[ic // 2]
                base = (ic % 2) * W
                w1t = w13t[:, bass.ds(base, NH * P)]
                w3t = w13t[:, bass.ds(base + NH * P, NH * P)]
                for nt in range(NCT):
                    cs = bass.ts(nt, CT)
                    p1 = ps1.tile([P, CT], F32, tag="p1")
                    p3 = ps3.tile([P, CT], F32, tag="p3")
                    for hc in range(NH):
                        nc.tensor.matmul(
                            p1[:],
                            lhsT=w1t[:, bass.ts(hc, P)],
                            rhs=xt[:, bass.ds(hc * C + nt * CT, CT)],
                            start=(hc == 0),
                            stop=(hc == NH - 1),
                        )
                    for hc in range(NH):
                        nc.tensor.matmul(
                            p3[:],
                            lhsT=w3t[:, bass.ts(hc, P)],
                            rhs=xt[:, bass.ds(hc * C + nt * CT, CT)],
                            start=(hc == 0),
                            stop=(hc == NH - 1),
                        )
                    # Balance the PSUM eviction: ACT does silu(p1), DVE
                    # copies p3 (one PSUM port, one PE wait) — ACT alone
                    # otherwise releases every PSUM slot and runs late.
                    # The two-wait mul is handled by _split_excess_waits.
                    s1 = act_pool.tile([P, CT], F16, tag="s1")
                    nc.scalar.activation(
                        s1[:], p1[:], mybir.ActivationFunctionType.Silu
                    )
                    c3 = act_pool.tile([P, CT], F16, tag="c3")
                    nc.vector.tensor_copy(c3[:], p3[:])
                    nc.vector.tensor_mul(hid_sb[ic][:, cs], s1[:], c3[:])

            # Stage 2: outT[hc] = w2 @ hid, contracting over I.
            for hc in range(NH):
                w2t = w2_sb[hc // 2]
                w2base = (hc % 2) * NI * P
                for nt in range(NCT):
                    cs = bass.ts(nt, CT)
                    po = pso.tile([P, CT], F32, tag="po")
                    for ic in range(NI):
                        nc.tensor.matmul(
                            po[:],
                            lhsT=w2t[:, bass.ds(w2base + ic * P, P)],
                            rhs=hid_sb[ic][:, cs],
                            start=(ic == 0),
                            stop=(ic == NI - 1),
                        )
                    # Evict split by PARTITION halves: two copies (DVE +
                    # ACT) and two 2KB-line DMAs (sync + scalar) all run in
                    # parallel, halving the exposed tail after the final
                    # matmul. Column splits would shrink the DMA lines below
                    # 2KB and hit the packet-rate wall instead.
                    # Single full-width eviction (fp32 rows = 2KB DMA lines);
                    # finer splits add end-of-program semaphore barriers that
                    # cost more than the overlap they buy.
                    ot = out_pool.tile([P, CT], F32, tag="ot")
                    nc.vector.tensor_copy(ot[:], po[:])
                    eng = nc.sync if hc % 2 == 0 else nc.scalar
                    eng.dma_start(
                        out=outT[hc * P:(hc + 1) * P, bass.ds(nt * CT, CT)],
                        in_=ot[:],
                    )
    _split_excess_waits(nc)
    return nc


def _split_excess_waits(nc, max_inline=1):
    """This walrus build rejects instructions carrying more than one inline
    sem wait ("Too many sync wait commands"). Move excess on_wait entries
    onto standalone InstEventSemaphore ops right before the instruction on
    the same engine (semantically identical: the engine stalls either way).
    """
    for blk in nc.m.functions[0].blocks:
        insts = blk.instructions
        out = []
        changed = False
        for inst in insts:
            si = inst.sync_info
            waits = list(si.on_wait) if si is not None and si.on_wait else []
            if len(waits) > max_inline and not isinstance(
                inst, mybir.InstEventSemaphore
            ):
                excess, keep = waits[:-max_inline], waits[-max_inline:]
                for k, w in enumerate(excess):
                    out.append(
                        mybir.InstEventSemaphore(
                            name=f"{inst.name}-evw{k}",
                            engine=inst.engine,
                            sync_info=mybir.SyncInfo(on_wait=[w], on_update=[]),
                        )
                    )
                inst.sync_info = mybir.SyncInfo(
                    on_wait=keep, on_update=list(si.on_update or [])
                )
                changed = True
            out.append(inst)
        if changed:
            blk.instructions = out
    return nc


def _route(x, gate_w):
    """Replicate the reference router in f64-stable numpy: returns
    (top_idx [T,K], top_w [T,K]) with renormalized weights."""
    logits = x.astype(np.float64) @ gate_w.astype(np.float64).T  # [T, E]
    m = logits.max(axis=-1, keepdims=True)
    p = np.exp(logits - m)
    p /= p.sum(axis=-1, keepdims=True)
    # top-2, ties broken by lower index (matches jax.lax.top_k)
    order = np.argsort(-p, axis=-1, kind="stable")
    top_i = order[:, :TOP_K]
    top_p = np.take_along_axis(p, top_i, axis=-1)
    top_w = top_p / top_p.sum(axis=-1, keepdims=True)
    return top_i, top_w.astype(np.float32)


def kernel(hidden_states, gate_w, w1, w2, w3):
    b, s, h = hidden_states.shape
    x = np.ascontiguousarray(
        np.asarray(hidden_states, dtype=np.float32).reshape(-1, h)
    )
    gate_w = np.asarray(gate_w, dtype=np.float32)
    w1 = np.asarray(w1, dtype=np.float32)
    w2 = np.asarray(w2, dtype=np.float32)
    w3 = np.asarray(w3, dtype=np.float32)

    top_i, top_w = _route(x, gate_w)

    # token lists per expert
    expert_rows = [np.where((top_i == e).any(axis=1))[0] for e in range(E)]
    # (row in expert buffer) for each (token, k) assignment
    in_maps = []
    overflow = []  # (e, token_idx, weight) handled on host
    gathers = []
    for e in range(E):
        rows = expert_rows[e]
        if len(rows) > C:
            keep = rows[:C]
            for t in rows[C:]:
                kk = np.where(top_i[t] == e)[0][0]
                overflow.append((e, int(t), float(top_w[t, kk])))
            rows = keep
        gathers.append(rows)
        xe = np.zeros((C, H), dtype=np.float32)
        xe[: len(rows)] = x[rows]
        # [P, NH*C]: xP[p, hc*C+c] = xe[c, hc*P+p]
        xP = np.ascontiguousarray(
            xe.T.reshape(NH, P, C).transpose(1, 0, 2).reshape(P, NH * C)
        ).astype(np.float16)
        w1c = w1[e].reshape(NI, P, NH, P).transpose(0, 3, 2, 1).reshape(NI, P, NH * P)
        w3c = w3[e].reshape(NI, P, NH, P).transpose(0, 3, 2, 1).reshape(NI, P, NH * P)
        w13c = np.concatenate([w1c, w3c], axis=2)  # [NI, P, 2*NH*P]
        # pair-pack: two consecutive ic blocks share a DRAM row (8KB lines)
        w13p = np.ascontiguousarray(
            w13c.reshape(NI // 2, 2, P, 2 * NH * P)
            .transpose(0, 2, 1, 3)
            .reshape(NI // 2, P, 4 * NH * P)
        ).astype(np.float16)
        w2c = (
            w2[e].reshape(NH, P, NI, P).transpose(0, 3, 2, 1).reshape(NH, P, NI * P)
        )
        # two hc blocks per DRAM row (halves DMA count, lines stay 7KB)
        w2c = np.ascontiguousarray(
            w2c.reshape(NH // 2, 2, P, NI * P)
            .transpose(0, 2, 1, 3)
            .reshape(NH // 2, P, 2 * NI * P)
        ).astype(np.float16)
        in_maps.append({"xP": xP, "w13p": w13p, "w2c": w2c})

    if "nc" not in _cache:
        _cache["nc"] = _build_moe_mlp()
    nc = _cache["nc"]

    res = run_bass_kernel_spmd(
        nc,
        in_maps,
        core_ids=list(range(E)),
        trace=bool(int(os.environ.get("MOE_TRACE", "0"))),
    )
    _cache["last_result"] = res

    out = np.zeros((T, H), dtype=np.float32)
    for e in range(E):
        rows = gathers[e]
        ye = np.ascontiguousarray(res.results[e]["outT"].T)[: len(rows)]  # [n_e, H]
        # routing weight of expert e for each routed token
        kidx = (top_i[rows] == e).argmax(axis=1)
        wts = top_w[rows, kidx][:, None]
        np.add.at(out, rows, ye * wts)

    if overflow:
        from collections import defaultdict
        by_e = defaultdict(list)
        for e, t, wt in overflow:
            by_e[e].append((t, wt))
        for e, lst in by_e.items():
            ts = np.array([t for t, _ in lst])
            wts = np.array([w for _, w in lst], dtype=np.float32)[:, None]
            xb = x[ts]
            hid = _silu_np(xb @ w1[e].T) * (xb @ w3[e].T)
            np.add.at(out, ts, wts * (hid @ w2[e].T))

    return out.reshape(b, s, h)


def _silu_np(v):
    return v / (1.0 + np.exp(-v))


# revision 60
# speedup vs baseline: 1.1971x; 1.0411x over previous
"""Mixtral sparse MoE block on 8 TRN2 NeuronCores.

Strategy (expert-parallel, per sharding hint):
  - Router (tiny: 2048x1024 @ 1024x8 + softmax + top-2) runs on host as part
    of the sharding step; it determines which tokens go to which core.
  - Core e holds expert e's weights (w1/w2/w3 in fp16, 22 MB) and receives
    the tokens routed to expert e (zero-padded to a static capacity C),
    pre-transposed and cast to fp16.
  - Device computes hidT = silu(W1 x^T) * (W3 x^T); outT = W2 hidT, i.e. the
    full SwiGLU MLP in transposed layout. fp16 operands, fp32 PSUM
    accumulation (matmul rate is identical to fp32r at N=512, but weight
    DMA and ldweights traffic halve).
  - Host scales each expert output row by its routing weight and scatter-adds
    back into the [T, H] output.

Shapes are hardcoded for the graded problem:
  hidden_states [1, 2048, 1024], gate_w [8, 1024],
  w1/w3 [8, 3584, 1024], w2 [8, 1024, 3584], fp32.
"""

import os

import numpy as np

import concourse.bass as bass
import concourse.tile as tile
from concourse import mybir
from concourse.bass_utils import run_bass_kernel_spmd

E = 8          # experts == cores
TOP_K = 2
H = 1024       # hidden
I = 3584       # intermediate
T = 2048       # tokens
P = 128
NH = H // P    # 8
NI = I // P    # 28
C = 512        # per-expert token capacity; overflow tokens go to the host path
CT = 512       # matmul N-tile == C: one full-width matmul per group
NCT = C // CT

F32 = mybir.dt.float32
F16 = mybir.dt.float16

_cache = {}


def _build_moe_mlp():
    """One-expert SwiGLU MLP, SPMD on 8 cores.

    Inputs (per core, host pre-arranged, fp16):
      xP   [P, NH*C]       tokens^T partition-packed: xP[p, hc*C+c] =
                           x[c-th routed token][hc*P+p]. One wide SBUF tile
                           (8 KB per partition line) so the DMA runs at
                           queue bandwidth instead of the 1KB-packet rate.
      w13p [NI/2, P, 2*2*NH*P] w1 and w3 block-packed, TWO ic blocks per
                           DRAM row (8KB lines — a 4KB-line stream loses the
                           saturated-fabric packet arbitration and starves):
                           w13p[ic//2, hp, (ic%2)*2*NH*P + h*P+ip] =
                             w1[ic*P+ip, h*P+hp]; w3 likewise at +NH*P.
      w2c  [NH, P, NI*P]   w2c[hc, ip, ic*P+hp] = w2[hc*P+hp, ic*P+ip]
    Output:
      outT [H, C] = (silu(x@w1.T) * (x@w3.T)) @ w2.T, transposed, fp32
    """
    nc = bass.Bass(use_seq_codegen=True)
    xP = nc.declare_dram_parameter("xP", [P, NH * C], F16, isOutput=False)
    w13p = nc.declare_dram_parameter(
        "w13p", [NI // 2, P, 4 * NH * P], F16, isOutput=False
    )
    w2c = nc.declare_dram_parameter("w2c", [NH, P, NI * P], F16, isOutput=False)
    outT = nc.declare_dram_parameter("outT", [H, C], F32, isOutput=True)

    with tile.TileContext(nc) as tc:
        with (
            tc.tile_pool(name="x_pool", bufs=1) as x_pool,
            tc.tile_pool(name="hid_pool", bufs=1) as hid_pool,
            tc.tile_pool(name="w13_pool", bufs=4) as w13_pool,
            tc.tile_pool(name="w2_pool", bufs=1) as w2_pool,
            tc.tile_pool(name="ps1", bufs=2, space="PSUM") as ps1,
            tc.tile_pool(name="ps3", bufs=2, space="PSUM") as ps3,
            tc.tile_pool(name="pso", bufs=2, space="PSUM") as pso,
            tc.tile_pool(name="act_pool", bufs=3) as act_pool,
            tc.tile_pool(name="out_pool", bufs=3) as out_pool,
        ):
            # Stage 0: token activations as ONE wide tile [P, NH*C].
            # DMA arbitration is per-PACKET (one partition line) round-robin
            # across queues, so throughput is proportional to line size:
            # keep every line >= 4KB. x: two half DMAs (4KB lines).
            # Startup: x quarters alternate the two HW queues in consumption
            # order; ic0 streams as four quarter-DMAs on gpsimd so the first
            # p1 chain only waits for ic0's w1 block, then ic1 follows as
            # one 4KB-line single.
            W = 2 * NH * P  # w13 columns per ic block
            p0 = w13_pool.tile([P, 2 * W], F16, tag="w13", name="w13p0")
            w13_tiles = {0: p0}

            xt = x_pool.tile([P, NH * C], F16, tag="x", name="x")
            XQ = NH * C // 4
            for q in range(4):
                eng = nc.sync if q % 2 == 0 else nc.scalar
                eng.dma_start(
                    out=xt[:, bass.ds(q * XQ, XQ)],
                    in_=xP[:, bass.ds(q * XQ, XQ)],
                )
            for qq in range(2):
                nc.gpsimd.dma_start(
                    out=p0[:, bass.ds(qq * W // 2, W // 2)],
                    in_=w13p[0, :, bass.ds(qq * W // 2, W // 2)],
                )
            nc.gpsimd.dma_start(out=p0[:, W:], in_=w13p[0, :, W:])


            # Queue plan (total DRAM BW ~350 GB/s is SHARED): the w13 stream
            # needs ~147 GB/s sustained and gets the fast software-DGE queue
            # (gpsimd, ~200 GB/s) to itself. x (1 MB now) and the w2
            # prefetch (7.3 MB, needed only at ~100us) share the two slower
            # HW queues (~70 GB/s each) — w2 lands by ~65us.
            w2_sb = []
            for hc in range(NH):
                w2t = w2_pool.tile([P, NI * P], F16, tag=f"w2_{hc}", name=f"w2_{hc}")
                nc.sync.dma_start(out=w2t[:], in_=w2c[hc, :, :])
                w2_sb.append(w2t)

            # hidT [I, C] lives in SBUF between the two stages (fp16).
            hid_sb = [
                hid_pool.tile([P, C], F16, tag=f"hid{ic}", name=f"hid{ic}") for ic in range(NI)
            ]

            # Stage 1: hidT[ic] = silu(p1) * p3, contracting over H.
            # w13 streams as one 8KB-line DMA per ic PAIR, pairs alternating
            # between gpsimd and scalar: the two queues together earn ~2/3
            # of the byte share, which the ~150 GB/s stream needs; sync
            # carries x + w2.
            for ic in range(NI):
                if ic % 2 == 0:
                    icp = ic // 2
                    if icp > 0:
                        w13t = w13_pool.tile([P, 2 * W], F16, tag="w13")
                        w13_tiles[icp] = w13t
                        eng = nc.gpsimd if icp % 2 == 0 else nc.scalar
                        eng.dma_start(out=w13t[:], in_=w13p[icp, :, :])
                    else:
                        w13t = w13_tiles[icp]
                else:
                    w13t = w13_tiles[ic // 2]
                base = (ic % 2) * W
                w1t = w13t[:, bass.ds(base, NH * P)]
                w3t = w13t[:, bass.ds(base + NH * P, NH * P)]
                for nt in range(NCT):
                    cs = bass.ts(nt, CT)
                    p1 = ps1.tile([P, CT], F32, tag="p1")
                    p3 = ps3.tile([P, CT], F32, tag="p3")
                    for hc in range(NH):
                        nc.tensor.matmul(
                            p1[:],
                            lhsT=w1t[:, bass.ts(hc, P)],
                            rhs=xt[:, bass.ds(hc * C + nt * CT, CT)],
                            start=(hc == 0),
                            stop=(hc == NH - 1),
                        )
                    for hc in range(NH):
                        nc.tensor.matmul(
                            p3[:],
                            lhsT=w3t[:, bass.ts(hc, P)],
                            rhs=xt[:, bass.ds(hc * C + nt * CT, CT)],
                            start=(hc == 0),
                            stop=(hc == NH - 1),
                        )
                    # Balance the PSUM eviction: ACT does silu(p1), DVE
                    # copies p3 (one PSUM port, one PE wait) — ACT alone
                    # otherwise releases every PSUM slot and runs late.
                    # The two-wait mul is handled by _split_excess_waits.
                    s1 = act_pool.tile([P, CT], F16, tag="s1")
                    nc.scalar.activation(
                        s1[:], p1[:], mybir.ActivationFunctionType.Silu
                    )
                    c3 = act_pool.tile([P, CT], F16, tag="c3")
                    nc.vector.tensor_copy(c3[:], p3[:])
                    nc.vector.tensor_mul(hid_sb[ic][:, cs], s1[:], c3[:])

            # Stage 2: outT[hc] = w2 @ hid, contracting over I.
            for hc in range(NH):
                w2t = w2_sb[hc]
                for nt in range(NCT):
                    cs = bass.ts(nt, CT)
                    po = pso.tile([P, CT], F32, tag="po")
                    for ic in range(NI):
                        nc.tensor.matmul(
                            po[:],
                            lhsT=w2t[:, bass.ts(ic, P)],
                            rhs=hid_sb[ic][:, cs],
                            start=(ic == 0),
                            stop=(ic == NI - 1),
                        )
                    # Evict split by PARTITION halves: two copies (DVE +
                    # ACT) and two 2KB-line DMAs (sync + scalar) all run in
                    # parallel, halving the exposed tail after the final
                    # matmul. Column splits would shrink the DMA lines below
                    # 2KB and hit the packet-rate wall instead.
                    # Single full-width eviction (fp32 rows = 2KB DMA lines);
                    # finer splits add end-of-program semaphore barriers that
                    # cost more than the overlap they buy.
                    ot = out_pool.tile([P, CT], F32, tag="ot")
                    nc.vector.tensor_copy(ot[:], po[:])
                    eng = nc.sync if hc % 2 == 0 else nc.scalar
                    eng.dma_start(
                        out=outT[hc * P:(hc + 1) * P, bass.ds(nt * CT, CT)],
                        in_=ot[:],
                    )
    _split_excess_waits(nc)
    return nc


def _split_excess_waits(nc, max_inline=1):
    """This walrus build rejects instructions carrying more than one inline
    sem wait ("Too many sync wait commands"). Move excess on_wait entries
    onto standalone InstEventSemaphore ops right before the instruction on
    the same engine (semantically identical: the engine stalls either way).
    """
    for blk in nc.m.functions[0].blocks:
        insts = blk.instructions
        out = []
        changed = False
        for inst in insts:
            si = inst.sync_info
            waits = list(si.on_wait) if si is not None and si.on_wait else []
            if len(waits) > max_inline and not isinstance(
                inst, mybir.InstEventSemaphore
            ):
                excess, keep = waits[:-max_inline], waits[-max_inline:]
                for k, w in enumerate(excess):
                    out.append(
                        mybir.InstEventSemaphore(
                            name=f"{inst.name}-evw{k}",
                            engine=inst.engine,
                            sync_info=mybir.SyncInfo(on_wait=[w], on_update=[]),
                        )
                    )
                inst.sync_info = mybir.SyncInfo(
                    on_wait=keep, on_update=list(si.on_update or [])
                )
                changed = True
            out.append(inst)
        if changed:
            blk.instructions = out
    return nc


def _route(x, gate_w):
    """Replicate the reference router in f64-stable numpy: returns
    (top_idx [T,K], top_w [T,K]) with renormalized weights."""
    logits = x.astype(np.float64) @ gate_w.astype(np.float64).T  # [T, E]
    m = logits.max(axis=-1, keepdims=True)
    p = np.exp(logits - m)
    p /= p.sum(axis=-1, keepdims=True)
    # top-2, ties broken by lower index (matches jax.lax.top_k)
    order = np.argsort(-p, axis=-1, kind="stable")
    top_i = order[:, :TOP_K]
    top_p = np.take_along_axis(p, top_i, axis=-1)
    top_w = top_p / top_p.sum(axis=-1, keepdims=True)
    return top_i, top_w.astype(np.float32)


def kernel(hidden_states, gate_w, w1, w2, w3):
    b, s, h = hidden_states.shape
    x = np.ascontiguousarray(
        np.asarray(hidden_states, dtype=np.float32).reshape(-1, h)
    )
    gate_w = np.asarray(gate_w, dtype=np.float32)
    w1 = np.asarray(w1, dtype=np.float32)
    w2 = np.asarray(w2, dtype=np.float32)
    w3 = np.asarray(w3, dtype=np.float32)

    top_i, top_w = _route(x, gate_w)

    # token lists per expert
    expert_rows = [np.where((top_i == e).any(axis=1))[0] for e in range(E)]
    # (row in expert buffer) for each (token, k) assignment
    in_maps = []
    overflow = []  # (e, token_idx, weight) handled on host
    gathers = []
    for e in range(E):
        rows = expert_rows[e]
        if len(rows) > C:
            keep = rows[:C]
            for t in rows[C:]:
                kk = np.where(top_i[t] == e)[0][0]
                overflow.append((e, int(t), float(top_w[t, kk])))
            rows = keep
        gathers.append(rows)
        xe = np.zeros((C, H), dtype=np.float32)
        xe[: len(rows)] = x[rows]
        # [P, NH*C]: xP[p, hc*C+c] = xe[c, hc*P+p]
        xP = np.ascontiguousarray(
            xe.T.reshape(NH, P, C).transpose(1, 0, 2).reshape(P, NH * C)
        ).astype(np.float16)
        w1c = w1[e].reshape(NI, P, NH, P).transpose(0, 3, 2, 1).reshape(NI, P, NH * P)
        w3c = w3[e].reshape(NI, P, NH, P).transpose(0, 3, 2, 1).reshape(NI, P, NH * P)
        w13c = np.concatenate([w1c, w3c], axis=2)  # [NI, P, 2*NH*P]
        # pair-pack: two consecutive ic blocks share a DRAM row (8KB lines)
        w13p = np.ascontiguousarray(
            w13c.reshape(NI // 2, 2, P, 2 * NH * P)
            .transpose(0, 2, 1, 3)
            .reshape(NI // 2, P, 4 * NH * P)
        ).astype(np.float16)
        w2c = np.ascontiguousarray(
            w2[e].reshape(NH, P, NI, P).transpose(0, 3, 2, 1).reshape(NH, P, NI * P)
        ).astype(np.float16)
        in_maps.append({"xP": xP, "w13p": w13p, "w2c": w2c})

    if "nc" not in _cache:
        _cache["nc"] = _build_moe_mlp()
    nc = _cache["nc"]

    res = run_bass_kernel_spmd(
        nc,
        in_maps,
        core_ids=list(range(E)),
        trace=bool(int(os.environ.get("MOE_TRACE", "0"))),
    )
    _cache["last_result"] = res

    out = np.zeros((T, H), dtype=np.float32)
    for e in range(E):
        rows = gathers[e]
        ye = np.ascontiguousarray(res.results[e]["outT"].T)[: len(rows)]  # [n_e, H]
        # routing weight of expert e for each routed token
        kidx = (top_i[rows] == e).argmax(axis=1)
        wts = top_w[rows, kidx][:, None]
        np.add.at(out, rows, ye * wts)

    if overflow:
        from collections import defaultdict
        by_e = defaultdict(list)
        for e, t, wt in overflow:
            by_e[e].append((t, wt))
        for e, lst in by_e.items():
            ts = np.array([t for t, _ in lst])
            wts = np.array([w for _, w in lst], dtype=np.float32)[:, None]
            xb = x[ts]
            hid = _silu_np(xb @ w1[e].T) * (xb @ w3[e].T)
            np.add.at(out, ts, wts * (hid @ w2[e].T))

    return out.reshape(b, s, h)


def _silu_np(v):
    return v / (1.0 + np.exp(-v))
